# revision 17
# baseline (speedup 1.0000x reference)
"""Multi-head attention (B=2, S=4096, D=1024, H=16) on 8 NeuronCores.

Sharding: core c = (batch b = c // 4, head-group g = c % 4).  Each head-group
owns 4 heads = 256 projection features.

v3: fp16 operands everywhere (single-pass PE matmuls, 10-bit mantissa keeps
rel err ~1e-3); all transposes/casts done on the HOST (inputs ship as
qT/kT/vT [D, S] fp16, wqT/wkT/wvT [D, E], w0T [E, D]); batched 3D-AP DMA
loads; softmax normalization fused into the PSUM->SBUF eviction of the PV
accumulator (reciprocal of the ones-column row, partition-broadcast,
tensor_tensor multiply) so there is no transpose-based normalize phase; the
output projection for a q-block is emitted right after its 4 heads finish,
overlapping the next block's attention.  Host sums the 4 fp16 partials per
batch in fp32.
"""

import numpy as np
from contextlib import ExitStack

import concourse.bass as bass
import concourse.bacc as bacc
import concourse.tile as tile
from concourse import mybir, bass_utils

B, S, D, H = 2, 4096, 1024, 16
DK = D // H          # 64
NCORES = 8
GROUPS = 4           # head-groups (tensor parallel)
HG = H // GROUPS     # 4 heads per group
E = HG * DK          # 256 features per group

F32 = mybir.dt.float32
F16 = mybir.dt.float16

P = 128              # partitions
SC = S // P          # 32 s-chunks of 128
SG = 8               # s-groups in projection
SGW = S // SG        # 512
DC = D // P          # 8 d-chunks
QB = 1024            # q-block in attention
NQB = S // QB        # 4
QC = QB // P         # 8 q-chunks per block
NST = SC             # 32 k-stripes of 128
VW = DK + 1          # vp columns per head incl. ones column (65)
VPAD = 66            # padded per-head stride in vps tile


def kernel_body(tc, qT, kT, vT, wqT, wkT, wvT, w0T, out):
    nc = tc.nc
    ctx = ExitStack()
    with ctx:
        # persistent weights (pre-transposed on host; loaded on the scalar
        # engine's DGE queue so activation loads on sync run in parallel)
        w_pool = ctx.enter_context(tc.tile_pool(name="wsb", bufs=1))
        w0sb = w_pool.tile([P, 2, D], F16, tag="w0sb", name="w0sb")
        wqsb = w_pool.tile([P, DC, E], F16, tag="wqsb", name="wqsb")
        wksb = w_pool.tile([P, DC, E], F16, tag="wksb", name="wksb")
        wvsb = w_pool.tile([P, DC, E], F16, tag="wvsb", name="wvsb")
        nc.scalar.dma_start(out=wqsb,
                            in_=wqT.rearrange("(c p) e -> p c e", p=P))
        nc.scalar.dma_start(out=wksb,
                            in_=wkT.rearrange("(c p) e -> p c e", p=P))
        nc.scalar.dma_start(out=wvsb,
                            in_=wvT.rearrange("(c p) e -> p c e", p=P))
        nc.scalar.dma_start(out=w0sb,
                            in_=w0T.rearrange("(c p) d -> p c d", p=P))

        proj_pool = ctx.enter_context(tc.tile_pool(name="proj", bufs=1))
        qpT = [proj_pool.tile([P, S], F16, tag=f"qpT{i}", name=f"qpT{i}")
               for i in range(2)]
        kpT = [proj_pool.tile([P, S], F16, tag=f"kpT{i}", name=f"kpT{i}")
               for i in range(2)]
        vps = proj_pool.tile([P, SC, HG * VPAD], F16, tag="vps", name="vps")

        xw_pool = ctx.enter_context(
            tc.tile_pool(name="xw", bufs=1, side="right"))
        xw = [xw_pool.tile([P, S], F16, tag=f"xw{i}", name=f"xw{i}")
              for i in range(2)]

        qTr = qT.rearrange("(c p) s -> p c s", p=P)
        kTr = kT.rearrange("(c p) s -> p c s", p=P)
        vTr = vT.rearrange("(c p) s -> p c s", p=P)

        # attention-side pools live for the whole kernel
        att_pool = ctx.enter_context(tc.tile_pool(name="a_att", bufs=3))
        rn_pool = ctx.enter_context(tc.tile_pool(name="a_rn", bufs=1))
        ppool_st = ctx.enter_context(
            tc.tile_pool(name="a_st", bufs=2, space="PSUM"))
        ppool_x = ctx.enter_context(
            tc.tile_pool(name="a_x", bufs=1, space="PSUM"))

        def att_iter(h, q0, kk, xacc):
            et, hp = h // 2, (h % 2) * DK
            attst = att_pool.tile([P, QB], F16, tag="att", name="att")
            lhs_k = kpT[et][hp:hp + DK, kk * P:(kk + 1) * P]
            st = ppool_st.tile([P, QB], F32, tag="st", name="st")
            for j in range(2):
                nc.tensor.matmul(
                    st[:, j * 512:(j + 1) * 512],
                    lhs_k,
                    qpT[et][hp:hp + DK, q0 + j * 512:q0 + (j + 1) * 512],
                    start=True, stop=True)
            nc.scalar.activation(
                attst, st, mybir.ActivationFunctionType.Exp, scale=0.125)
            lhs_v = vps[:, kk, h * VPAD:h * VPAD + VW]
            for j in range(2):
                nc.tensor.matmul(
                    xacc[:, j * 512:(j + 1) * 512],
                    lhs_v,
                    attst[:, j * 512:(j + 1) * 512],
                    start=(kk == 0), stop=(kk == NST - 1))

        def normalize(h, q0, xacc):
            # divide the PV accumulator rows by the ones-column rowsum and
            # write into xw in the natural [e, q] layout the out-proj needs
            et, hp = h // 2, (h % 2) * DK
            rcp = rn_pool.tile([1, QB], F32, tag="rcp", name="rcp")
            nc.vector.reciprocal(rcp, xacc[DK:DK + 1, :])
            rbc = rn_pool.tile([DK, QB], F32, tag="rbc", name="rbc")
            nc.gpsimd.partition_broadcast(rbc, rcp)
            nc.vector.tensor_tensor(
                xw[et][hp:hp + DK, q0:q0 + QB],
                xacc[0:DK, :], rbc, mybir.AluOpType.mult)

        GW = 2 * SGW            # 1024-column projection groups
        NG = S // GW            # 4

        # ===== phase P fused with attention (h=0, qb=0) =====
        # attention consumes k-stripes in order, so each projected k/v group
        # immediately feeds 8 attention iterations while the next group's
        # projection runs; the ACT engine starts ~80us earlier than a
        # sequential projection phase would allow
        with tc.tile_pool(name="p_xin", bufs=2) as xin_pool, \
             tc.tile_pool(name="p_psum", bufs=1, space="PSUM") as ppool_a:
            ones_sc = xin_pool.tile([P, SC], F16, tag="ones_sc",
                                    name="ones_sc")
            nc.vector.memset(ones_sc, 1.0)

            def qproj(g):
                s0 = g * GW
                qg = xin_pool.tile([P, DC, GW], F16, tag="qg", name="qg")
                nc.gpsimd.dma_start(out=qg, in_=qTr[:, :, s0:s0 + GW])
                for half in range(2):
                    h0 = half * SGW
                    for et in range(2):
                        acc = ppool_a.tile([P, SGW], F32, tag="acc",
                                           name="acc")
                        for dc in range(DC):
                            nc.tensor.matmul(
                                acc,
                                wqsb[:, dc, et * P:(et + 1) * P],
                                qg[:, dc, h0:h0 + SGW],
                                start=(dc == 0), stop=(dc == DC - 1))
                        nc.vector.tensor_copy(
                            out=qpT[et][:, s0 + h0:s0 + h0 + SGW], in_=acc)

            qproj(0)
            xacc0 = ppool_x.tile([VW, QB], F32, tag="xacc", name="xacc")
            for g in range(NG):
                s0 = g * GW
                kg = xin_pool.tile([P, DC, GW], F16, tag="kg", name="kg")
                vg = xin_pool.tile([P, DC, GW], F16, tag="vg", name="vg")
                nc.sync.dma_start(out=kg, in_=kTr[:, :, s0:s0 + GW])
                nc.scalar.dma_start(out=vg, in_=vTr[:, :, s0:s0 + GW])
                for half in range(2):
                    h0 = half * SGW
                    for et in range(2):
                        acc = ppool_a.tile([P, SGW], F32, tag="acc",
                                           name="acc")
                        for dc in range(DC):
                            nc.tensor.matmul(
                                acc,
                                wksb[:, dc, et * P:(et + 1) * P],
                                kg[:, dc, h0:h0 + SGW],
                                start=(dc == 0), stop=(dc == DC - 1))
                        nc.vector.tensor_copy(
                            out=kpT[et][:, s0 + h0:s0 + h0 + SGW], in_=acc)
                    for sc4 in range(SGW // P):
                        scg = (s0 + h0) // P + sc4
                        accv = ppool_a.tile([P, E], F32, tag="accv",
                                            name="accv")
                        for dc in range(DC):
                            nc.tensor.matmul(
                                accv,
                                vg[:, dc, h0 + sc4 * P:h0 + (sc4 + 1) * P],
                                wvsb[:, dc, :],
                                start=(dc == 0), stop=(dc == DC - 1))
                        for h in range(HG):
                            nc.vector.tensor_copy(
                                out=vps[:, scg, h * VPAD:h * VPAD + DK],
                                in_=accv[:, h * DK:(h + 1) * DK])
                g8 = g * (GW // P)
                for h in range(HG):
                    col = h * VPAD + DK
                    nc.vector.tensor_copy(
                        out=vps[:, g8:g8 + GW // P, col:col + 1],
                        in_=ones_sc[:, g8:g8 + GW // P].rearrange(
                            "p (s o) -> p s o", o=1))
                for kk in range(g8, g8 + GW // P):
                    att_iter(0, 0, kk, xacc0)
            for g in range(1, NG):
                qproj(g)
            normalize(0, 0, xacc0)

        # ===== rest of attention + interleaved out-projection =====
        with tc.tile_pool(name="a_osb", bufs=2) as osb_pool, \
             tc.tile_pool(name="a_w", bufs=2, space="PSUM") as ppool_w:

            def emit_wproj(qb, qc):
                qq = qb * QB + qc * P
                osb = osb_pool.tile([P, D], F16, tag="osb", name="osb")
                for j in range(2):
                    oacc = ppool_w.tile([P, 512], F32, tag="oacc",
                                        name="oacc")
                    for ec in range(2):
                        nc.tensor.matmul(
                            oacc,
                            xw[ec][:, qq:qq + P],
                            w0sb[:, ec, j * 512:(j + 1) * 512],
                            start=(ec == 0), stop=(ec == 1))
                    nc.vector.tensor_copy(
                        out=osb[:, j * 512:(j + 1) * 512], in_=oacc)
                nc.sync.dma_start(out=out[qq:qq + P, :], in_=osb)

            for qb in range(NQB):
                q0 = qb * QB
                for h in range(HG):
                    if qb == 0 and h == 0:
                        continue
                    xacc = ppool_x.tile([VW, QB], F32, tag="xacc",
                                        name="xacc")
                    for kk in range(NST):
                        att_iter(h, q0, kk, xacc)
                        # spread the previous block's out-projection across
                        # k-stripes instead of one burst that would leave
                        # the ACT engine idle
                        if h == 0 and kk % 4 == 3:
                            emit_wproj(qb - 1, kk // 4)
                    normalize(h, q0, xacc)
            for qc in range(QC):
                emit_wproj(NQB - 1, qc)


def build_program():
    nc = bacc.Bacc("TRN2", target_bir_lowering=False, debug=False,
                   num_devices=NCORES)
    qT = nc.dram_tensor("qT", (D, S), F16, kind="ExternalInput").ap()
    kT = nc.dram_tensor("kT", (D, S), F16, kind="ExternalInput").ap()
    vT = nc.dram_tensor("vT", (D, S), F16, kind="ExternalInput").ap()
    wqT = nc.dram_tensor("wqT", (D, E), F16, kind="ExternalInput").ap()
    wkT = nc.dram_tensor("wkT", (D, E), F16, kind="ExternalInput").ap()
    wvT = nc.dram_tensor("wvT", (D, E), F16, kind="ExternalInput").ap()
    w0T = nc.dram_tensor("w0T", (E, D), F16, kind="ExternalInput").ap()
    out = nc.dram_tensor("out", (S, D), F16, kind="ExternalOutput").ap()
    with tile.TileContext(nc) as tc:
        kernel_body(tc, qT, kT, vT, wqT, wkT, wvT, w0T, out)
    nc.compile()
    return nc


_NC_CACHE = None


def _get_program():
    global _NC_CACHE
    if _NC_CACHE is None:
        _NC_CACHE = build_program()
    return _NC_CACHE


def make_in_maps(q, k, v, wq, wk, wv, w0):
    arrs = [np.asarray(a, dtype=np.float32)
            for a in (q, k, v, wq, wk, wv, w0)]
    q, k, v, wq, wk, wv, w0 = arrs
    f16 = np.float16
    # per-batch transposed activations (shared by the 4 cores of a batch)
    qTb = [np.ascontiguousarray(q[b].T).astype(f16) for b in range(B)]
    kTb = [np.ascontiguousarray(k[b].T).astype(f16) for b in range(B)]
    vTb = [np.ascontiguousarray(v[b].T).astype(f16) for b in range(B)]
    in_maps = []
    for c in range(NCORES):
        b, g = c // GROUPS, c % GROUPS
        e0 = g * E
        in_maps.append({
            "qT": qTb[b],
            "kT": kTb[b],
            "vT": vTb[b],
            "wqT": np.ascontiguousarray(wq[e0:e0 + E, :].T).astype(f16),
            "wkT": np.ascontiguousarray(wk[e0:e0 + E, :].T).astype(f16),
            "wvT": np.ascontiguousarray(wv[e0:e0 + E, :].T).astype(f16),
            "w0T": np.ascontiguousarray(w0[:, e0:e0 + E].T).astype(f16),
        })
    return in_maps


def gather_out(results):
    out = np.zeros((B, S, D), dtype=np.float32)
    for c in range(NCORES):
        b = c // GROUPS
        out[b] += results[c]["out"].astype(np.float32)
    return out


def _install_ntff_hook_shim():
    """This image's antenv lacks axon_hooks; recreate it so trace=True works.

    Mirrors trn_agent_boot.trn_boot._ntff_profile_via_ctypes against
    /opt/axon/libaxon_pjrt.so.
    """
    import sys, types, ctypes, contextlib
    if "antenv.axon_hooks" in sys.modules:
        return
    mod = types.ModuleType("antenv.axon_hooks")
    mod._hook = None

    def set_axon_ntff_profile_hook(h):
        mod._hook = h

    def get_axon_ntff_profile_hook():
        return mod._hook

    mod.set_axon_ntff_profile_hook = set_axon_ntff_profile_hook
    mod.get_axon_ntff_profile_hook = get_axon_ntff_profile_hook
    sys.modules["antenv.axon_hooks"] = mod
    try:
        import antenv
        antenv.axon_hooks = mod
    except ImportError:
        pass

    so_path = "/opt/axon/libaxon_pjrt.so"
    try:
        lib = ctypes.CDLL(so_path)
        if not hasattr(lib, "axon_start_nrt_profile"):
            return
        lib.axon_start_nrt_profile.argtypes = [
            ctypes.POINTER(ctypes.c_int64), ctypes.c_size_t]
        lib.axon_start_nrt_profile.restype = ctypes.c_int64
        lib.axon_stop_nrt_profile.argtypes = [ctypes.c_char_p]
        lib.axon_stop_nrt_profile.restype = ctypes.c_int64
    except OSError:
        return

    @contextlib.contextmanager
    def _hook(output_dir, device_ids):
        import jax
        jax.devices()
        if device_ids:
            ids = (ctypes.c_int64 * len(device_ids))(*device_ids)
            rc = lib.axon_start_nrt_profile(ids, len(device_ids))
        else:
            rc = lib.axon_start_nrt_profile(None, 0)
        if rc != 0:
            raise RuntimeError(f"axon_start_nrt_profile rc={rc}")
        try:
            yield
        finally:
            n = lib.axon_stop_nrt_profile(str(output_dir).encode())
            print(f"profile: {n} file(s) written to {output_dir}")

    mod._hook = _hook


def kernel(q, k, v, wq, wk, wv, w0, _trace=False, _tmpdir=None):
    if _trace:
        _install_ntff_hook_shim()
    nc = _get_program()
    in_maps = make_in_maps(q, k, v, wq, wk, wv, w0)
    res = bass_utils.run_bass_kernel_spmd(
        nc, in_maps, core_ids=list(range(NCORES)),
        trace=_trace, tmpdir=_tmpdir)
    out = gather_out(res.results)
    if _trace:
        return out, res
    return out


# revision 18
# speedup vs baseline: 1.0419x; 1.0419x over previous
"""Multi-head attention (B=2, S=4096, D=1024, H=16) on 8 NeuronCores.

Sharding: core c = (batch b = c // 4, head-group g = c % 4).  Each head-group
owns 4 heads = 256 projection features.

v3: fp16 operands everywhere (single-pass PE matmuls, 10-bit mantissa keeps
rel err ~1e-3); all transposes/casts done on the HOST (inputs ship as
qT/kT/vT [D, S] fp16, wqT/wkT/wvT [D, E], w0T [E, D]); batched 3D-AP DMA
loads; softmax normalization fused into the PSUM->SBUF eviction of the PV
accumulator (reciprocal of the ones-column row, partition-broadcast,
tensor_tensor multiply) so there is no transpose-based normalize phase; the
output projection for a q-block is emitted right after its 4 heads finish,
overlapping the next block's attention.  Host sums the 4 fp16 partials per
batch in fp32.
"""

import numpy as np
from contextlib import ExitStack

import concourse.bass as bass
import concourse.bacc as bacc
import concourse.tile as tile
from concourse import mybir, bass_utils

B, S, D, H = 2, 4096, 1024, 16
DK = D // H          # 64
NCORES = 8
GROUPS = 4           # head-groups (tensor parallel)
HG = H // GROUPS     # 4 heads per group
E = HG * DK          # 256 features per group

F32 = mybir.dt.float32
F16 = mybir.dt.float16

P = 128              # partitions
SC = S // P          # 32 s-chunks of 128
SG = 8               # s-groups in projection
SGW = S // SG        # 512
DC = D // P          # 8 d-chunks
QB = 1024            # q-block in attention
NQB = S // QB        # 4
QC = QB // P         # 8 q-chunks per block
NST = SC             # 32 k-stripes of 128
VW = DK + 1          # vp columns per head incl. ones column (65)
VPAD = 66            # padded per-head stride in vps tile


def kernel_body(tc, qT, kT, vT, wqT, wkT, wvT, w0T, out):
    nc = tc.nc
    ctx = ExitStack()
    with ctx:
        # persistent weights (pre-transposed on host; loaded on the scalar
        # engine's DGE queue so activation loads on sync run in parallel)
        w_pool = ctx.enter_context(tc.tile_pool(name="wsb", bufs=1))
        w0sb = w_pool.tile([P, 2, D], F16, tag="w0sb", name="w0sb")
        wqsb = w_pool.tile([P, DC, E], F16, tag="wqsb", name="wqsb")
        wksb = w_pool.tile([P, DC, E], F16, tag="wksb", name="wksb")
        wvsb = w_pool.tile([P, DC, E], F16, tag="wvsb", name="wvsb")
        nc.scalar.dma_start(out=wqsb,
                            in_=wqT.rearrange("(c p) e -> p c e", p=P))
        nc.scalar.dma_start(out=wksb,
                            in_=wkT.rearrange("(c p) e -> p c e", p=P))
        nc.scalar.dma_start(out=wvsb,
                            in_=wvT.rearrange("(c p) e -> p c e", p=P))
        nc.scalar.dma_start(out=w0sb,
                            in_=w0T.rearrange("(c p) d -> p c d", p=P))

        proj_pool = ctx.enter_context(tc.tile_pool(name="proj", bufs=1))
        qpT = [proj_pool.tile([P, S], F16, tag=f"qpT{i}", name=f"qpT{i}")
               for i in range(2)]
        kpT = [proj_pool.tile([P, S], F16, tag=f"kpT{i}", name=f"kpT{i}")
               for i in range(2)]
        vps = proj_pool.tile([P, SC, HG * VPAD], F16, tag="vps", name="vps")

        xw_pool = ctx.enter_context(
            tc.tile_pool(name="xw", bufs=1, side="right"))
        xw = [xw_pool.tile([P, S], F16, tag=f"xw{i}", name=f"xw{i}")
              for i in range(2)]

        qTr = qT.rearrange("(c p) s -> p c s", p=P)
        kTr = kT.rearrange("(c p) s -> p c s", p=P)
        vTr = vT.rearrange("(c p) s -> p c s", p=P)

        # attention-side pools live for the whole kernel
        att_pool = ctx.enter_context(tc.tile_pool(name="a_att", bufs=4))
        rn_pool = ctx.enter_context(tc.tile_pool(name="a_rn", bufs=1))
        ppool_st = ctx.enter_context(
            tc.tile_pool(name="a_st", bufs=2, space="PSUM"))
        ppool_x = ctx.enter_context(
            tc.tile_pool(name="a_x", bufs=1, space="PSUM"))

        def att_iter(h, q0, kk, xacc):
            et, hp = h // 2, (h % 2) * DK
            attst = att_pool.tile([P, QB], F16, tag="att", name="att")
            lhs_k = kpT[et][hp:hp + DK, kk * P:(kk + 1) * P]
            st = ppool_st.tile([P, QB], F32, tag="st", name="st")
            for j in range(2):
                nc.tensor.matmul(
                    st[:, j * 512:(j + 1) * 512],
                    lhs_k,
                    qpT[et][hp:hp + DK, q0 + j * 512:q0 + (j + 1) * 512],
                    start=True, stop=True)
            nc.scalar.activation(
                attst, st, mybir.ActivationFunctionType.Exp, scale=0.125)
            lhs_v = vps[:, kk, h * VPAD:h * VPAD + VW]
            for j in range(2):
                nc.tensor.matmul(
                    xacc[:, j * 512:(j + 1) * 512],
                    lhs_v,
                    attst[:, j * 512:(j + 1) * 512],
                    start=(kk == 0), stop=(kk == NST - 1))

        def normalize(h, q0, xacc):
            # divide the PV accumulator rows by the ones-column rowsum and
            # write into xw in the natural [e, q] layout the out-proj needs
            et, hp = h // 2, (h % 2) * DK
            rcp = rn_pool.tile([1, QB], F32, tag="rcp", name="rcp")
            nc.vector.reciprocal(rcp, xacc[DK:DK + 1, :])
            rbc = rn_pool.tile([DK, QB], F32, tag="rbc", name="rbc")
            nc.gpsimd.partition_broadcast(rbc, rcp)
            nc.vector.tensor_tensor(
                xw[et][hp:hp + DK, q0:q0 + QB],
                xacc[0:DK, :], rbc, mybir.AluOpType.mult)

        GW = 2 * SGW            # 1024-column projection groups
        NG = S // GW            # 4

        # ===== phase P fused with attention (h=0, qb=0) =====
        # attention consumes k-stripes in order, so each projected k/v group
        # immediately feeds 8 attention iterations while the next group's
        # projection runs; the ACT engine starts ~80us earlier than a
        # sequential projection phase would allow
        with tc.tile_pool(name="p_xin", bufs=2) as xin_pool, \
             tc.tile_pool(name="p_psum", bufs=1, space="PSUM") as ppool_a:
            ones_sc = xin_pool.tile([P, SC], F16, tag="ones_sc",
                                    name="ones_sc")
            nc.vector.memset(ones_sc, 1.0)

            def qproj(g):
                s0 = g * GW
                qg = xin_pool.tile([P, DC, GW], F16, tag="qg", name="qg")
                nc.gpsimd.dma_start(out=qg, in_=qTr[:, :, s0:s0 + GW])
                for half in range(2):
                    h0 = half * SGW
                    for et in range(2):
                        acc = ppool_a.tile([P, SGW], F32, tag="acc",
                                           name="acc")
                        for dc in range(DC):
                            nc.tensor.matmul(
                                acc,
                                wqsb[:, dc, et * P:(et + 1) * P],
                                qg[:, dc, h0:h0 + SGW],
                                start=(dc == 0), stop=(dc == DC - 1))
                        nc.vector.tensor_copy(
                            out=qpT[et][:, s0 + h0:s0 + h0 + SGW], in_=acc)

            qproj(0)
            xacc0 = ppool_x.tile([VW, QB], F32, tag="xacc", name="xacc")
            for g in range(NG):
                s0 = g * GW
                kg = xin_pool.tile([P, DC, GW], F16, tag="kg", name="kg")
                vg = xin_pool.tile([P, DC, GW], F16, tag="vg", name="vg")
                nc.sync.dma_start(out=kg, in_=kTr[:, :, s0:s0 + GW])
                nc.scalar.dma_start(out=vg, in_=vTr[:, :, s0:s0 + GW])
                for half in range(2):
                    h0 = half * SGW
                    for et in range(2):
                        acc = ppool_a.tile([P, SGW], F32, tag="acc",
                                           name="acc")
                        for dc in range(DC):
                            nc.tensor.matmul(
                                acc,
                                wksb[:, dc, et * P:(et + 1) * P],
                                kg[:, dc, h0:h0 + SGW],
                                start=(dc == 0), stop=(dc == DC - 1))
                        nc.vector.tensor_copy(
                            out=kpT[et][:, s0 + h0:s0 + h0 + SGW], in_=acc)
                    for sc4 in range(SGW // P):
                        scg = (s0 + h0) // P + sc4
                        accv = ppool_a.tile([P, E], F32, tag="accv",
                                            name="accv")
                        for dc in range(DC):
                            nc.tensor.matmul(
                                accv,
                                vg[:, dc, h0 + sc4 * P:h0 + (sc4 + 1) * P],
                                wvsb[:, dc, :],
                                start=(dc == 0), stop=(dc == DC - 1))
                        for h in range(HG):
                            nc.vector.tensor_copy(
                                out=vps[:, scg, h * VPAD:h * VPAD + DK],
                                in_=accv[:, h * DK:(h + 1) * DK])
                g8 = g * (GW // P)
                for h in range(HG):
                    col = h * VPAD + DK
                    nc.vector.tensor_copy(
                        out=vps[:, g8:g8 + GW // P, col:col + 1],
                        in_=ones_sc[:, g8:g8 + GW // P].rearrange(
                            "p (s o) -> p s o", o=1))
                for kk in range(g8, g8 + GW // P):
                    att_iter(0, 0, kk, xacc0)
            for g in range(1, NG):
                qproj(g)
            normalize(0, 0, xacc0)

        # ===== rest of attention + interleaved out-projection =====
        with tc.tile_pool(name="a_osb", bufs=2) as osb_pool, \
             tc.tile_pool(name="a_w", bufs=2, space="PSUM") as ppool_w:

            def emit_wproj(qb, qc):
                qq = qb * QB + qc * P
                osb = osb_pool.tile([P, D], F16, tag="osb", name="osb")
                for j in range(2):
                    oacc = ppool_w.tile([P, 512], F32, tag="oacc",
                                        name="oacc")
                    for ec in range(2):
                        nc.tensor.matmul(
                            oacc,
                            xw[ec][:, qq:qq + P],
                            w0sb[:, ec, j * 512:(j + 1) * 512],
                            start=(ec == 0), stop=(ec == 1))
                    nc.vector.tensor_copy(
                        out=osb[:, j * 512:(j + 1) * 512], in_=oacc)
                nc.sync.dma_start(out=out[qq:qq + P, :], in_=osb)

            for qb in range(NQB):
                q0 = qb * QB
                for h in range(HG):
                    if qb == 0 and h == 0:
                        continue
                    xacc = ppool_x.tile([VW, QB], F32, tag="xacc",
                                        name="xacc")
                    for kk in range(NST):
                        att_iter(h, q0, kk, xacc)
                        # spread the previous block's out-projection across
                        # k-stripes instead of one burst that would leave
                        # the ACT engine idle
                        if h == 0 and kk % 4 == 3:
                            emit_wproj(qb - 1, kk // 4)
                    normalize(h, q0, xacc)
            for qc in range(QC):
                emit_wproj(NQB - 1, qc)


def build_program():
    nc = bacc.Bacc("TRN2", target_bir_lowering=False, debug=False,
                   num_devices=NCORES)
    qT = nc.dram_tensor("qT", (D, S), F16, kind="ExternalInput").ap()
    kT = nc.dram_tensor("kT", (D, S), F16, kind="ExternalInput").ap()
    vT = nc.dram_tensor("vT", (D, S), F16, kind="ExternalInput").ap()
    wqT = nc.dram_tensor("wqT", (D, E), F16, kind="ExternalInput").ap()
    wkT = nc.dram_tensor("wkT", (D, E), F16, kind="ExternalInput").ap()
    wvT = nc.dram_tensor("wvT", (D, E), F16, kind="ExternalInput").ap()
    w0T = nc.dram_tensor("w0T", (E, D), F16, kind="ExternalInput").ap()
    out = nc.dram_tensor("out", (S, D), F16, kind="ExternalOutput").ap()
    with tile.TileContext(nc) as tc:
        kernel_body(tc, qT, kT, vT, wqT, wkT, wvT, w0T, out)
    nc.compile()
    return nc


_NC_CACHE = None


def _get_program():
    global _NC_CACHE
    if _NC_CACHE is None:
        _NC_CACHE = build_program()
    return _NC_CACHE


def make_in_maps(q, k, v, wq, wk, wv, w0):
    arrs = [np.asarray(a, dtype=np.float32)
            for a in (q, k, v, wq, wk, wv, w0)]
    q, k, v, wq, wk, wv, w0 = arrs
    f16 = np.float16
    # per-batch transposed activations (shared by the 4 cores of a batch)
    qTb = [np.ascontiguousarray(q[b].T).astype(f16) for b in range(B)]
    kTb = [np.ascontiguousarray(k[b].T).astype(f16) for b in range(B)]
    vTb = [np.ascontiguousarray(v[b].T).astype(f16) for b in range(B)]
    in_maps = []
    for c in range(NCORES):
        b, g = c // GROUPS, c % GROUPS
        e0 = g * E
        in_maps.append({
            "qT": qTb[b],
            "kT": kTb[b],
            "vT": vTb[b],
            "wqT": np.ascontiguousarray(wq[e0:e0 + E, :].T).astype(f16),
            "wkT": np.ascontiguousarray(wk[e0:e0 + E, :].T).astype(f16),
            "wvT": np.ascontiguousarray(wv[e0:e0 + E, :].T).astype(f16),
            "w0T": np.ascontiguousarray(w0[:, e0:e0 + E].T).astype(f16),
        })
    return in_maps


def gather_out(results):
    out = np.zeros((B, S, D), dtype=np.float32)
    for c in range(NCORES):
        b = c // GROUPS
        out[b] += results[c]["out"].astype(np.float32)
    return out


def _install_ntff_hook_shim():
    """This image's antenv lacks axon_hooks; recreate it so trace=True works.

    Mirrors trn_agent_boot.trn_boot._ntff_profile_via_ctypes against
    /opt/axon/libaxon_pjrt.so.
    """
    import sys, types, ctypes, contextlib
    if "antenv.axon_hooks" in sys.modules:
        return
    mod = types.ModuleType("antenv.axon_hooks")
    mod._hook = None

    def set_axon_ntff_profile_hook(h):
        mod._hook = h

    def get_axon_ntff_profile_hook():
        return mod._hook

    mod.set_axon_ntff_profile_hook = set_axon_ntff_profile_hook
    mod.get_axon_ntff_profile_hook = get_axon_ntff_profile_hook
    sys.modules["antenv.axon_hooks"] = mod
    try:
        import antenv
        antenv.axon_hooks = mod
    except ImportError:
        pass

    so_path = "/opt/axon/libaxon_pjrt.so"
    try:
        lib = ctypes.CDLL(so_path)
        if not hasattr(lib, "axon_start_nrt_profile"):
            return
        lib.axon_start_nrt_profile.argtypes = [
            ctypes.POINTER(ctypes.c_int64), ctypes.c_size_t]
        lib.axon_start_nrt_profile.restype = ctypes.c_int64
        lib.axon_stop_nrt_profile.argtypes = [ctypes.c_char_p]
        lib.axon_stop_nrt_profile.restype = ctypes.c_int64
    except OSError:
        return

    @contextlib.contextmanager
    def _hook(output_dir, device_ids):
        import jax
        jax.devices()
        if device_ids:
            ids = (ctypes.c_int64 * len(device_ids))(*device_ids)
            rc = lib.axon_start_nrt_profile(ids, len(device_ids))
        else:
            rc = lib.axon_start_nrt_profile(None, 0)
        if rc != 0:
            raise RuntimeError(f"axon_start_nrt_profile rc={rc}")
        try:
            yield
        finally:
            n = lib.axon_stop_nrt_profile(str(output_dir).encode())
            print(f"profile: {n} file(s) written to {output_dir}")

    mod._hook = _hook


def kernel(q, k, v, wq, wk, wv, w0, _trace=False, _tmpdir=None):
    if _trace:
        _install_ntff_hook_shim()
    nc = _get_program()
    in_maps = make_in_maps(q, k, v, wq, wk, wv, w0)
    res = bass_utils.run_bass_kernel_spmd(
        nc, in_maps, core_ids=list(range(NCORES)),
        trace=_trace, tmpdir=_tmpdir)
    out = gather_out(res.results)
    if _trace:
        return out, res
    return out
